# revision 1
# baseline (speedup 1.0000x reference)
"""MoE expert-routing kernel for Trainium2 (8 NeuronCores).

out[b] = x[b] @ weight[index[b]] + bias[index[b]]

Expert-parallel sharding (4 experts/core), host-side token routing
(stable argsort, capacity C per expert), fp16 operands/output with fp32
PSUM accumulation. Transposed compute layout — weights stationary,
tokens moving:

out^T[o, t] = sum_i W[i, o] * xT[i, t] + b[o], per expert, computed as
2 o-half PSUM groups x 2 K-half matmuls with N = C tokens (C=192 < 256),
25% fewer streamed PE rows than the token-stationary layout. Bias is a
per-partition column -> DVE tensor_scalar_add during the PSUM->SBUF move.

Host-packed fp16 block per expert ([128, 4*128 + 2 + 2C]):
  blk[e] = [w(k0,o0) | w(k0,o1) | w(k1,o0) | w(k1,o1) | b_o0 b_o1 | xT_h0 | xT_h1]
Output [EPC, 128, 2, C] fp16 (o_half-partitioned), untransposed on host.
"""

import numpy as np

B, E, DIN, DOUT = 4096, 32, 256, 256
NCORES = 8
EPC = E // NCORES

TRACE = False
LAST_RESULT = None

_PROGRAM_CACHE = {}


def _build_program(C):
    import concourse.bass as bass
    import concourse.mybir as mybir
    import concourse.tile as tile
    from concourse import bacc

    f32 = mybir.dt.float32
    f16 = mybir.dt.float16

    W = 4 * 128 + 2 + 2 * C
    boff = 4 * 128
    xoff = boff + 2
    CK = 512                  # token chunk per PSUM group (f32 bank limit)

    nc = bacc.Bacc("TRN2", target_bir_lowering=False, debug=False,
                   enable_asserts=False)

    blk_d = nc.dram_tensor("blk", [EPC, 128, W], f16, kind="ExternalInput")
    bc_d = nc.dram_tensor("bcol", [128, EPC * 2], f32, kind="ExternalInput")
    out_d = nc.dram_tensor("out", [EPC, 128, 2, C], f16,
                           kind="ExternalOutput")

    with tile.TileContext(nc) as tc:
        with (
            tc.tile_pool(name="bin", bufs=4) as bpool,
            tc.tile_pool(name="oout", bufs=4) as opool,
            tc.tile_pool(name="psum", bufs=6, space=bass.MemorySpace.PSUM)
                as ppool,
        ):
            bct = bpool.tile([128, EPC * 2], f32, tag="bcol")
            nc.gpsimd.dma_start(bct[:], bc_d.ap())
            blks = []
            for e in range(EPC):
                blk = bpool.tile([128, W], f16)
                eng = nc.sync if e % 2 == 0 else nc.scalar
                eng.dma_start(blk[:], blk_d.ap()[e])
                blks.append(blk)

            for e in range(EPC):
                blk = blks[e]
                ot = opool.tile([128, 2, C], f16)
                for oh in range(2):
                    for ck in range(0, C, CK):
                        cw = min(CK, C - ck)
                        ps = ppool.tile([128, CK], f32)
                        for k in range(2):
                            nc.tensor.matmul(
                                ps[:, :cw],
                                blk[:, (k * 2 + oh) * 128:
                                    (k * 2 + oh + 1) * 128],
                                blk[:, xoff + k * C + ck:
                                    xoff + k * C + ck + cw],
                                start=(k == 0), stop=(k == 1),
                            )
                        nc.vector.tensor_scalar_add(
                            ot[:, oh, ck:ck + cw], ps[:, :cw],
                            bct[:, e * 2 + oh:e * 2 + oh + 1])
                if e == EPC - 1:
                    nc.sync.dma_start(out_d.ap()[e][:, 0, :], ot[:, 0, :])
                    nc.scalar.dma_start(out_d.ap()[e][:, 1, :], ot[:, 1, :])
                else:
                    eng = nc.sync if e % 2 == 0 else nc.scalar
                    eng.dma_start(out_d.ap()[e], ot[:])

    nc.compile()
    return nc


def _route(index):
    order = np.argsort(index, kind="stable")
    counts = np.bincount(index, minlength=E)
    offs = np.zeros(E + 1, np.int64)
    offs[1:] = np.cumsum(counts)
    C = max(64, int(-(-int(counts.max()) // 64) * 64))
    return order, counts, offs, C


def _pack_core(x16, w16, b16, order, offs, C, c):
    W = 4 * 128 + 2 + 2 * C
    boff = 4 * 128
    xoff = boff + 2
    blk = np.zeros((EPC, 128, W), np.float16)
    for sl in range(EPC):
        e = c * EPC + sl
        toks = order[offs[e]:offs[e + 1]]
        xT = x16[toks].T
        for k in range(2):
            for oh in range(2):
                blk[sl, :, (k * 2 + oh) * 128:(k * 2 + oh + 1) * 128] = \
                    w16[e, k * 128:(k + 1) * 128, oh * 128:(oh + 1) * 128]
        blk[sl, :, boff] = b16[e, 0:128]
        blk[sl, :, boff + 1] = b16[e, 128:256]
        blk[sl, :, xoff:xoff + xT.shape[1]] = xT[0:128]
        blk[sl, :, xoff + C:xoff + C + xT.shape[1]] = xT[128:256]
    return np.ascontiguousarray(blk)


def kernel(x, index, weight, bias):
    from concourse.bass_utils import run_bass_kernel_spmd

    global LAST_RESULT

    x = np.asarray(x, np.float32)
    index = np.asarray(index, np.int32)
    weight = np.asarray(weight, np.float32)
    bias = np.asarray(bias, np.float32)

    order, counts, offs, C = _route(index)

    if C not in _PROGRAM_CACHE:
        _PROGRAM_CACHE[C] = _build_program(C)
    nc = _PROGRAM_CACHE[C]

    x16 = x.astype(np.float16)
    w16 = weight.astype(np.float16)
    b16 = bias.astype(np.float16)
    in_maps = []
    for c in range(NCORES):
        bcol = bias[c * EPC:(c + 1) * EPC].reshape(EPC * 2, 128).T
        in_maps.append({
            "blk": _pack_core(x16, w16, b16, order, offs, C, c),
            "bcol": np.ascontiguousarray(bcol, np.float32),
        })

    kwargs = {}
    if TRACE:
        kwargs = dict(trace=True, trace_cores=list(range(NCORES)))
    res = run_bass_kernel_spmd(nc, in_maps, core_ids=list(range(NCORES)),
                               **kwargs)
    LAST_RESULT = res

    out = np.empty((B, DOUT), np.float32)
    for c in range(NCORES):
        oc = res.results[c]["out"]  # [EPC, 128, 2, C] fp16
        for sl in range(EPC):
            e = c * EPC + sl
            toks = order[offs[e]:offs[e + 1]]
            oe = oc[sl].transpose(2, 1, 0).reshape(C, DOUT)
            out[toks] = oe[:len(toks)].astype(np.float32)
    return out

